# revision 10
# baseline (speedup 1.0000x reference)
"""TRN2 Bass kernel for nn_MaxRollingMeanAttentionProbe (sparse_attention).

Computation (reference):
    y      = relu(x @ w1 + b1)                    # [n, 256]
    logits = y @ queries.T ; vals = y @ values.T  # [n, 16]
    window i of size w: score_i = sum_j softmax(logits[i:i+w])_j * vals[i:i+w]_j
    out    = sum_h max_i score[i, h]              # scalar

Strategy: data-parallel over tokens across 8 NeuronCores with a recomputed
(w-1)-token halo, so no collectives are needed (the softmax shift cancels
exactly within any window).

Per core (one SPMD Tile program):
  pass A: stream host pre-transposed fp8e4 x in PAIRED 2 MiB DMAs (two
          512-token groups per trigger; bigger descriptor chains sustain
          higher HBM rate and halve the serial DMA-trigger load on the sync
          engine).  fp8 DoubleRow matmuls (2 k-tiles per instruction, 2x
          fp16 FLOP rate on HW) accumulate yT in fp32 PSUM; relu with
          scale=1/16 folds away the host-side x16 scaling of w1 (its 0.02
          magnitudes would otherwise sit in fp8e4's subnormal range and
          quantize at 2e-3 steps).  queries/values are similarly scaled x16
          and the scale is undone in the exp (logits) and on the host
          (vals).  A combined [queries; values] lhsT yields logits+vals
          stacked in one [32, 512] PSUM tile per group; probe work runs one
          PAIR behind the MLP.  Each probe result is copied PSUM->SBUF
          relay (DVE) and DMA'd into the per-column pass-B tiles from the
          GPSIMD software-DGE queue (DMA has no partition-base alignment
          restriction; engine ops need base % 32 == 0, and the sync engine
          is saturated issuing x loads).  Groups run in column-major order
          with the trailing halo group first, so each pass-B slab's column
          tiles close early.
  pass B: layout [128 partitions = 8 subchunks x 16 heads, tokens] split
          into GPS=4 per-column tiles + a (w-1)-wide halo tile, processed
          as 4 token slabs; slab t only depends on columns t and t+1, so
          its DVE/ScalarE work overlaps the tail of pass A instead of
          serializing after it.  Per slab: exp on ScalarE (scale=1/16);
          sliding-window sums via the DVE hardware prefix scan (fp32
          state) + shifted subtract; scores = Wsum/Z with a fast
          approximate reciprocal; max-reduce -> [128, 5].
Host: pack/scale/cast inputs (fp8e4 for matmul operands), final max/sum
      (tiny) including the /16 vals descale.
"""

import numpy as np

# Problem constants (shapes are fixed by the problem spec).
N_TOKENS = 131072
D_MODEL = 2048
D_HID = 256
N_HEADS = 16
N_CORES = 8
P = 128                    # SBUF partitions
G = 512                    # tokens per matmul group
TPC = N_TOKENS // N_CORES  # window starts per core (16384)
GPC = TPC // G             # groups per core without halo (32)
NSUB = 8                   # subchunks per core in pass B
SUB = TPC // NSUB          # window starts per subchunk (2048)
ND = D_MODEL // P          # 16 d_model chunks
ND2 = ND // 2              # 8 DoubleRow k-tile pairs
NH2 = D_HID // P           # 2 hidden halves
GPS = SUB // G             # groups (columns) per subchunk = pass-B slabs (4)
NPAIR = GPC // 2           # paired body-group loads (16)
COL_ORDER = (0, 1, 2, 3)   # halo tile (written by col-0 groups) lands early
W1_SCALE = 16.0            # host pre-scale on w1 (fp8 subnormal avoidance)
QV_SCALE = 16.0            # host pre-scale on queries/values

_NC_CACHE = {}


def _build(w: int, mmdt: str = "f8"):
    import concourse.bacc as bacc
    import concourse.tile as tile
    from concourse import mybir
    from contextlib import ExitStack

    F32 = mybir.dt.float32
    MDT = {"f8": mybir.dt.float8e4, "f16": mybir.dt.float16}[mmdt]
    DR = mybir.MatmulPerfMode.DoubleRow if mmdt == "f8" else None
    AF = mybir.ActivationFunctionType
    AX = mybir.AxisListType

    NG = -(-(TPC + w - 1) // G)    # groups per core incl. halo (33 for w>1)
    SUBLEN = SUB + w - 1           # tokens per subchunk
    SPLIT = SUB - w + 1            # j < SPLIT -> valid everywhere
    HALO = NG > GPC                # trailing halo-only group exists
    LW = min(G, ((w - 1 + 63) // 64) * 64) if HALO else G
    SPLITL = SPLIT - (GPS - 1) * G  # SPLIT within the last slab (G - w + 1)
    HP = ((w - 1 + 15) // 16 * 16) if w > 1 else 16  # halo tile width

    w1_scale = W1_SCALE if mmdt == "f8" else 1.0
    qv_scale = QV_SCALE if mmdt == "f8" else 1.0

    nc = bacc.Bacc(
        "TRN2",
        target_bir_lowering=False,
        debug=False,
        enable_asserts=False,
        num_devices=N_CORES,
    )
    # Paired body groups in column-major processing order:
    # xp[j, :, g2] = body group perm[2j + g2] (see _prep_inputs).
    xp = nc.dram_tensor("xp", [NPAIR, P, 2, ND, G], MDT, kind="ExternalInput")
    if HALO:
        xh = nc.dram_tensor("xh", [P, ND, LW], MDT, kind="ExternalInput")
    w1p = nc.dram_tensor("w1p", [P, ND, D_HID], MDT, kind="ExternalInput")
    b1p = nc.dram_tensor("b1p", [P, NH2], F32, kind="ExternalInput")
    # Combined probe weights [k, hh, m]: columns 0..15 = queries, 16..31 =
    # values -> one matmul yields logits/vals stacked in PSUM rows 0..31.
    qvp = nc.dram_tensor("qvp", [P, NH2, 2 * N_HEADS], MDT, kind="ExternalInput")
    res = nc.dram_tensor("res", [P, GPS + 1], F32, kind="ExternalOutput")

    with tile.TileContext(nc) as tc, ExitStack() as ctx:
        const = ctx.enter_context(tc.tile_pool(name="const", bufs=1))
        w1_sb = const.tile([P, ND, D_HID], MDT)
        for q4 in range(4):
            nq = ND // 4
            nc.sync.dma_start(
                out=w1_sb[:, q4 * nq : (q4 + 1) * nq, :],
                in_=w1p[:, q4 * nq : (q4 + 1) * nq, :],
            )
        b1_sb = const.tile([P, NH2], F32)
        nc.sync.dma_start(out=b1_sb[:], in_=b1p[:])
        qv_sb = const.tile([P, NH2, 2 * N_HEADS], MDT)
        nc.sync.dma_start(out=qv_sb[:], in_=qvp[:])

        # Pass-B layout, split per column c: partition s*16+h, free dim =
        # token c*G+j of subchunk s.  Separate tiles keep the dependency
        # tracking column-granular so pass-B slabs start mid-stream.
        bp = ctx.enter_context(tc.tile_pool(name="bp", bufs=1))
        RLc = [bp.tile([P, G], F32, name=f"RLc{c}") for c in range(GPS)]
        RVc = [bp.tile([P, G], F32, name=f"RVc{c}") for c in range(GPS)]
        RLH = bp.tile([P, HP], F32)
        RVH = bp.tile([P, HP], F32)

        xpool = ctx.enter_context(tc.tile_pool(name="xpool", bufs=5))
        ypool = ctx.enter_context(tc.tile_pool(name="ypool", bufs=6))
        rpool = ctx.enter_context(tc.tile_pool(name="rpool", bufs=4))
        spool = ctx.enter_context(tc.tile_pool(name="spool", bufs=2))
        psy = ctx.enter_context(tc.tile_pool(name="psy", bufs=4, space="PSUM"))
        pslv = ctx.enter_context(tc.tile_pool(name="pslv", bufs=3, space="PSUM"))

        # ---------------- pass A: MLP + probes ----------------
        def emit_mlp(xsl, gw, ytile, hh):
            """One hidden half: 8 DoubleRow (or 16 fp16) matmuls + relu.
            xsl(a, b) -> the [128, b-a, gw] slice of this group's x tile."""
            ypt = psy.tile([P, gw], F32, tag="ypsum")
            if DR is not None:
                for d2 in range(ND2):
                    nc.tensor.matmul(
                        ypt[:],
                        w1_sb[:, 2 * d2 : 2 * d2 + 2, hh * P : (hh + 1) * P],
                        xsl(2 * d2, 2 * d2 + 2),
                        start=(d2 == 0),
                        stop=(d2 == ND2 - 1),
                        perf_mode=DR,
                    )
            else:
                for d in range(ND):
                    nc.tensor.matmul(
                        ypt[:],
                        w1_sb[:, d, hh * P : (hh + 1) * P],
                        xsl(d, d + 1),
                        start=(d == 0),
                        stop=(d == ND - 1),
                    )
            nc.scalar.activation(
                ytile[:, hh, :], ypt[:], AF.Relu,
                bias=b1_sb[:, hh : hh + 1], scale=1.0 / w1_scale,
            )

        def emit_probe(g, gw, ytile):
            lv = pslv.tile([2 * N_HEADS, gw], F32, tag="lvp")
            if DR is not None:
                nc.tensor.matmul(
                    lv[:], qv_sb[:, 0:NH2, :], ytile[:, 0:NH2, :],
                    start=True, stop=True, perf_mode=DR,
                )
            else:
                for hh in range(NH2):
                    nc.tensor.matmul(
                        lv[:], qv_sb[:, hh, :], ytile[:, hh, :],
                        start=(hh == 0), stop=(hh == NH2 - 1),
                    )
            rl = rpool.tile([2 * N_HEADS, gw], F32, tag="relay")
            nc.vector.tensor_copy(out=rl[:], in_=lv[:])
            if HALO and g == NG - 1:
                # trailing halo-only group: subchunk NSUB-1's halo tokens
                h0 = (NSUB - 1) * N_HEADS
                nc.sync.dma_start(
                    out=RLH[h0 : h0 + N_HEADS, 0 : w - 1],
                    in_=rl[0:N_HEADS, 0 : w - 1],
                )
                nc.scalar.dma_start(
                    out=RVH[h0 : h0 + N_HEADS, 0 : w - 1],
                    in_=rl[N_HEADS : 2 * N_HEADS, 0 : w - 1],
                )
                return
            s, c = g // GPS, g % GPS
            rlo = s * N_HEADS
            nc.sync.dma_start(
                out=RLc[c][rlo : rlo + N_HEADS, 0:gw], in_=rl[0:N_HEADS, :]
            )
            nc.scalar.dma_start(
                out=RVc[c][rlo : rlo + N_HEADS, 0:gw],
                in_=rl[N_HEADS : 2 * N_HEADS, :],
            )
            if c == 0 and s > 0 and w > 1:
                h0 = (s - 1) * N_HEADS
                nc.sync.dma_start(
                    out=RLH[h0 : h0 + N_HEADS, 0 : w - 1],
                    in_=rl[0:N_HEADS, 0 : w - 1],
                )
                nc.scalar.dma_start(
                    out=RVH[h0 : h0 + N_HEADS, 0 : w - 1],
                    in_=rl[N_HEADS : 2 * N_HEADS, 0 : w - 1],
                )

        # Column-major pair order: all subchunks' column c before column
        # c+1, so pass-B slab t unblocks once column t+1 lands; the tiny
        # halo group runs first (it is independent and warms up the PE).
        pairs = [
            (2 * i * GPS + c, (2 * i + 1) * GPS + c)
            for c in COL_ORDER
            for i in range(NSUB // 2)
        ]
        pending = []
        if HALO:
            xt = xpool.tile([P, ND, LW], MDT, tag="xh")
            nc.sync.dma_start(out=xt[:], in_=xh[:])
            yt = ypool.tile([P, NH2, LW], MDT, tag="yt")
            for hh in range(NH2):
                emit_mlp(lambda a, b: xt[:, a:b, :], LW, yt, hh)
            pending.append((NG - 1, LW, yt))
        for j, pr in enumerate(pairs):
            xt = xpool.tile([P, 2, ND, G], MDT, tag="xt")
            if j <= 1:
                # Split the first paired load 4 ways so the PE fills sooner.
                nq = ND // 4
                for q4 in range(4):
                    nc.sync.dma_start(
                        out=xt[:, :, q4 * nq : (q4 + 1) * nq, :],
                        in_=xp[j, :, :, q4 * nq : (q4 + 1) * nq, :],
                    )
            else:
                # one chain per group: group A's MLP unblocks on its own 1
                # MiB chain, and two chains sustain a higher aggregate rate
                for g2 in range(2):
                    nc.sync.dma_start(
                        out=xt[:, g2, :, :], in_=xp[j, :, g2, :, :]
                    )
            for g2, g in enumerate(pr):
                yt = ypool.tile([P, NH2, G], MDT, tag="yt")
                for hh in range(NH2):
                    emit_mlp(
                        lambda a, b, g2=g2, xt=xt: xt[:, g2, a:b, :], G, yt, hh
                    )
                pending.append((g, G, yt))
            limit = 0 if (j + 1) % (NSUB // 2) == 0 else 4
            while len(pending) > limit:
                emit_probe(*pending.pop(0))
        for pnd in pending:
            emit_probe(*pnd)

        # ---------------- pass B: windowed softmax-mean scores ----------------
        # exp directly (no max shift: the shift cancels exactly within each
        # window and logits are O(1), far from the f32 exp overflow bound);
        # sliding-window sums via the DVE prefix scan: Z[j] = cs[j+w] - cs[j].
        # Slab t reads column tile t plus the first w-1 tokens of column
        # t+1 (or the halo tile), so it overlaps the tail of pass A.
        ADD = mybir.AluOpType.add
        BYP = mybir.AluOpType.bypass
        smax = bp.tile([P, GPS + 1], F32)
        for t in range(GPS):
            L = G + w - 1              # tokens this slab reads
            LP = (L + 16 + 15) // 16 * 16
            extL = RLc[t + 1] if t < GPS - 1 else RLH
            extV = RVc[t + 1] if t < GPS - 1 else RVH
            E = spool.tile([P, LP], F32, tag="E")
            EV = spool.tile([P, LP], F32, tag="EV")
            csZ = spool.tile([P, LP], F32, tag="csZ")
            csW = spool.tile([P, LP], F32, tag="csW")
            # main part: needs only column t.  Z chain on DVE, W chain on
            # GPSIMD so the two prefix scans run concurrently.
            nc.scalar.activation(
                E[:, 0:G], RLc[t][:, :], AF.Exp, scale=1.0 / qv_scale
            )
            nc.gpsimd.tensor_mul(EV[:, 0:G], E[:, 0:G], RVc[t][:, :])
            nc.vector.memset(csZ[:, 0:1], 0.0)
            nc.vector.memset(csW[:, 0:1], 0.0)
            nc.vector.tensor_tensor_scan(
                out=csZ[:, 1 : 1 + G], data0=E[:, 0:G], data1=E[:, 0:G],
                initial=0.0, op0=ADD, op1=BYP,
            )
            nc.vector.tensor_tensor_scan(
                out=csW[:, 1 : 1 + G], data0=EV[:, 0:G], data1=EV[:, 0:G],
                initial=0.0, op0=ADD, op1=BYP,
            )
            if w > 1:
                # ext part: the first w-1 tokens of column t+1 (or the halo
                # tile), folded in late via scan chaining (initial=carry).
                nc.scalar.activation(
                    E[:, G:L], extL[:, 0 : w - 1], AF.Exp,
                    scale=1.0 / qv_scale,
                )
                nc.gpsimd.tensor_mul(
                    EV[:, G:L], E[:, G:L], extV[:, 0 : w - 1]
                )
                nc.vector.tensor_tensor_scan(
                    out=csZ[:, 1 + G : 1 + L], data0=E[:, G:L],
                    data1=E[:, G:L], initial=csZ[:, G : G + 1],
                    op0=ADD, op1=BYP,
                )
                nc.vector.tensor_tensor_scan(
                    out=csW[:, 1 + G : 1 + L], data0=EV[:, G:L],
                    data1=EV[:, G:L], initial=csW[:, G : G + 1],
                    op0=ADD, op1=BYP,
                )
            # Z -> E, Wn -> EV (slab inputs are dead after the scans)
            nc.gpsimd.tensor_sub(
                out=E[:, 0:G], in0=csZ[:, w : w + G], in1=csZ[:, 0:G]
            )
            nc.gpsimd.tensor_sub(
                out=EV[:, 0:G], in0=csW[:, w : w + G], in1=csW[:, 0:G]
            )
            nc.vector.reciprocal_approx_fast(out=csZ[:, 0:G], in_=E[:, 0:G])
            nc.vector.tensor_mul(
                out=csW[:, 0:G], in0=EV[:, 0:G], in1=csZ[:, 0:G]
            )
            if t < GPS - 1:
                nc.vector.reduce_max(
                    out=smax[:, t : t + 1], in_=csW[:, 0:G], axis=AX.X
                )
            else:
                if SPLITL > 0:
                    nc.vector.reduce_max(
                        out=smax[:, t : t + 1], in_=csW[:, 0:SPLITL],
                        axis=AX.X,
                    )
                else:
                    nc.vector.memset(smax[:, t : t + 1], -3.0e38)
                if SPLITL < G:
                    nc.vector.reduce_max(
                        out=smax[:, t + 1 : t + 2],
                        in_=csW[:, SPLITL:G], axis=AX.X,
                    )
                else:
                    nc.vector.memset(smax[:, t + 1 : t + 2], -3.0e38)
        nc.sync.dma_start(out=res[:], in_=smax[:])

    nc.compile()
    return nc


MM_DTYPE = "f8"


def _get_nc(w: int):
    key = (w, MM_DTYPE)
    nc = _NC_CACHE.get(key)
    if nc is None:
        nc = _build(w, MM_DTYPE)
        _NC_CACHE[key] = nc
    return nc


def _mm_cast(a: np.ndarray) -> np.ndarray:
    """Convert to the matmul input dtype (host-side rounding)."""
    import ml_dtypes

    if MM_DTYPE == "f8":
        return a.astype(ml_dtypes.float8_e4m3)
    return a.astype(np.float16)


def _prep_inputs(x, w1, b1, queries, values, w):
    """Host-side packing: pad + scale + round + transpose into DMA-friendly
    layouts. Returns the per-core in_maps for run_bass_kernel_spmd."""
    NG = -(-(TPC + w - 1) // G)
    HALO = NG > GPC
    LW = min(G, ((w - 1 + 63) // 64) * 64) if HALO else G
    NGG = (N_CORES - 1) * GPC + NG  # distinct global groups incl. final halo
    w1_scale = W1_SCALE if MM_DTYPE == "f8" else 1.0
    qv_scale = QV_SCALE if MM_DTYPE == "f8" else 1.0
    xpad = np.zeros((NGG * G, D_MODEL), dtype=np.float32)
    xpad[:N_TOKENS] = x
    xr = _mm_cast(xpad)
    # [gg, p, d, t] = xpad[gg*G + t, d*128 + p]
    xg_all = np.ascontiguousarray(
        xr.reshape(NGG, G, ND, P).transpose(0, 3, 2, 1)
    )
    # Column-major pair permutation matching _build's processing order.
    perm = [
        g
        for c in COL_ORDER
        for i in range(NSUB // 2)
        for g in (2 * i * GPS + c, (2 * i + 1) * GPS + c)
    ]
    w1p = np.ascontiguousarray(
        _mm_cast(w1 * w1_scale).reshape(ND, P, D_HID).transpose(1, 0, 2)
    )
    b1p = np.ascontiguousarray(np.asarray(b1, np.float32).reshape(NH2, P).T)
    # Combined probe weights: [k, hh, m] with columns 0..15 = queries.T
    # chunk, columns 16..31 = values.T chunk.
    qv = np.concatenate(
        [np.asarray(queries, np.float32), np.asarray(values, np.float32)], axis=0
    )  # [32, 256]
    qvT = _mm_cast(qv * qv_scale).T.reshape(NH2, P, 2 * N_HEADS)  # [hh, k, m]
    qvp = np.ascontiguousarray(qvT.transpose(1, 0, 2))
    in_maps = []
    for c in range(N_CORES):
        body = xg_all[c * GPC : c * GPC + GPC][perm]  # [32, P, ND, G]
        m = {
            "xp": np.ascontiguousarray(
                body.reshape(NPAIR, 2, P, ND, G).transpose(0, 2, 1, 3, 4)
            ),
            "w1p": w1p,
            "b1p": b1p,
            "qvp": qvp,
        }
        if HALO:
            m["xh"] = np.ascontiguousarray(xg_all[c * GPC + NG - 1, :, :, 0:LW])
        in_maps.append(m)
    return in_maps


def _combine(results, w):
    """Host-side final reduction: per-core [128, GPS+1] -> scalar."""
    qv_scale = QV_SCALE if MM_DTYPE == "f8" else 1.0
    best = np.full(N_HEADS, -np.inf, dtype=np.float64)
    for c in range(N_CORES):
        r = np.asarray(results[c]["res"], dtype=np.float64).reshape(
            NSUB, N_HEADS, GPS + 1
        )
        if c == N_CORES - 1 and w >= 2:
            r = r.copy()
            r[NSUB - 1, :, GPS] = -np.inf  # windows past n - w on last core
        best = np.maximum(best, r.max(axis=(0, 2)))
    return np.asarray(best.sum() / qv_scale, dtype=np.float32)


def kernel(x, w1, b1, queries, values, window_size):
    from concourse.bass_utils import run_bass_kernel_spmd

    x = np.asarray(x, dtype=np.float32)
    w1 = np.asarray(w1, dtype=np.float32)
    b1 = np.asarray(b1, dtype=np.float32)
    queries = np.asarray(queries, dtype=np.float32)
    values = np.asarray(values, dtype=np.float32)
    w = int(np.asarray(window_size))
    assert x.shape == (N_TOKENS, D_MODEL), x.shape
    assert 1 <= w <= G + 1  # slab/halo duplication reads at most one column

    nc = _get_nc(w)
    in_maps = _prep_inputs(x, w1, b1, queries, values, w)
    last_err = None
    for attempt in range(3):
        try:
            out = run_bass_kernel_spmd(nc, in_maps, core_ids=list(range(N_CORES)))
            return _combine(out.results, w)
        except Exception as e:  # transient terminal/device failures
            last_err = e
            import time as _time

            _time.sleep(5.0 * (attempt + 1))
    raise last_err


# Optional: expose a traced run for profiling from test harnesses.
def kernel_traced(x, w1, b1, queries, values, window_size, tmpdir=None):
    from concourse.bass_utils import run_bass_kernel_spmd

    w = int(np.asarray(window_size))
    nc = _get_nc(w)
    in_maps = _prep_inputs(
        np.asarray(x, np.float32),
        np.asarray(w1, np.float32),
        np.asarray(b1, np.float32),
        np.asarray(queries, np.float32),
        np.asarray(values, np.float32),
        w,
    )
    out = run_bass_kernel_spmd(
        nc, in_maps, core_ids=list(range(N_CORES)), trace=True, tmpdir=tmpdir
    )
    return _combine(out.results, w), out


# revision 11
# speedup vs baseline: 1.1733x; 1.1733x over previous
"""TRN2 Bass kernel for nn_MaxRollingMeanAttentionProbe (sparse_attention).

Computation (reference):
    y      = relu(x @ w1 + b1)                    # [n, 256]
    logits = y @ queries.T ; vals = y @ values.T  # [n, 16]
    window i of size w: score_i = sum_j softmax(logits[i:i+w])_j * vals[i:i+w]_j
    out    = sum_h max_i score[i, h]              # scalar

Strategy: data-parallel over tokens across 8 NeuronCores with a recomputed
(w-1)-token halo, so no collectives are needed (the softmax shift cancels
exactly within any window).

Per core (one SPMD Tile program):
  pass A: stream host pre-transposed fp8e4 x in PAIRED 2 MiB DMAs (two
          512-token groups per trigger; bigger descriptor chains sustain
          higher HBM rate and halve the serial DMA-trigger load on the sync
          engine).  fp8 DoubleRow matmuls (2 k-tiles per instruction, 2x
          fp16 FLOP rate on HW) accumulate yT in fp32 PSUM; relu with
          scale=1/16 folds away the host-side x16 scaling of w1 (its 0.02
          magnitudes would otherwise sit in fp8e4's subnormal range and
          quantize at 2e-3 steps).  queries/values are similarly scaled x16
          and the scale is undone in the exp (logits) and on the host
          (vals).  A combined [queries; values] lhsT yields logits+vals
          stacked in one [32, 512] PSUM tile per group; probe work runs one
          PAIR behind the MLP.  Each probe result is copied PSUM->SBUF
          relay (DVE) and DMA'd into the per-column pass-B tiles from the
          GPSIMD software-DGE queue (DMA has no partition-base alignment
          restriction; engine ops need base % 32 == 0, and the sync engine
          is saturated issuing x loads).  Groups run in column-major order
          with the trailing halo group first, so each pass-B slab's column
          tiles close early.
  pass B: layout [128 partitions = 8 subchunks x 16 heads, tokens] split
          into GPS=4 per-column tiles + a (w-1)-wide halo tile, processed
          as 4 token slabs; slab t only depends on columns t and t+1, so
          its DVE/ScalarE work overlaps the tail of pass A instead of
          serializing after it.  Per slab: exp on ScalarE (scale=1/16);
          sliding-window sums via the DVE hardware prefix scan (fp32
          state) + shifted subtract; scores = Wsum/Z with a fast
          approximate reciprocal; max-reduce -> [128, 5].
Host: pack/scale/cast inputs (fp8e4 for matmul operands), final max/sum
      (tiny) including the /16 vals descale.
"""

import numpy as np

# Problem constants (shapes are fixed by the problem spec).
N_TOKENS = 131072
D_MODEL = 2048
D_HID = 256
N_HEADS = 16
N_CORES = 8
P = 128                    # SBUF partitions
G = 512                    # tokens per matmul group
TPC = N_TOKENS // N_CORES  # window starts per core (16384)
GPC = TPC // G             # groups per core without halo (32)
NSUB = 8                   # subchunks per core in pass B
SUB = TPC // NSUB          # window starts per subchunk (2048)
ND = D_MODEL // P          # 16 d_model chunks
ND2 = ND // 2              # 8 DoubleRow k-tile pairs
NH2 = D_HID // P           # 2 hidden halves
GPS = SUB // G             # groups (columns) per subchunk = pass-B slabs (4)
NPAIR = GPC // 2           # paired body-group loads (16)
COL_ORDER = (0, 1, 2, 3)   # halo tile (written by col-0 groups) lands early
W1_SCALE = 16.0            # host pre-scale on w1 (fp8 subnormal avoidance)
QV_SCALE = 16.0            # host pre-scale on queries/values

_NC_CACHE = {}


def _build(w: int, mmdt: str = "f8"):
    import concourse.bacc as bacc
    import concourse.tile as tile
    from concourse import mybir
    from contextlib import ExitStack

    F32 = mybir.dt.float32
    MDT = {"f8": mybir.dt.float8e4, "f16": mybir.dt.float16}[mmdt]
    DR = mybir.MatmulPerfMode.DoubleRow if mmdt == "f8" else None
    AF = mybir.ActivationFunctionType
    AX = mybir.AxisListType

    NG = -(-(TPC + w - 1) // G)    # groups per core incl. halo (33 for w>1)
    SUBLEN = SUB + w - 1           # tokens per subchunk
    SPLIT = SUB - w + 1            # j < SPLIT -> valid everywhere
    HALO = NG > GPC                # trailing halo-only group exists
    LW = min(G, ((w - 1 + 63) // 64) * 64) if HALO else G
    SPLITL = SPLIT - (GPS - 1) * G  # SPLIT within the last slab (G - w + 1)
    HP = ((w - 1 + 15) // 16 * 16) if w > 1 else 16  # halo tile width

    w1_scale = W1_SCALE if mmdt == "f8" else 1.0
    qv_scale = QV_SCALE if mmdt == "f8" else 1.0

    nc = bacc.Bacc(
        "TRN2",
        target_bir_lowering=False,
        debug=False,
        enable_asserts=False,
        num_devices=N_CORES,
    )
    # Paired body groups in column-major processing order:
    # xp[j, :, g2] = body group perm[2j + g2] (see _prep_inputs).
    xp = nc.dram_tensor("xp", [NPAIR, P, 2, ND, G], MDT, kind="ExternalInput")
    if HALO:
        xh = nc.dram_tensor("xh", [P, ND, LW], MDT, kind="ExternalInput")
    w1p = nc.dram_tensor("w1p", [P, ND, D_HID], MDT, kind="ExternalInput")
    b1p = nc.dram_tensor("b1p", [P, NH2], F32, kind="ExternalInput")
    # Combined probe weights [k, hh, m]: columns 0..15 = queries, 16..31 =
    # values -> one matmul yields logits/vals stacked in PSUM rows 0..31.
    qvp = nc.dram_tensor("qvp", [P, NH2, 2 * N_HEADS], MDT, kind="ExternalInput")
    res = nc.dram_tensor("res", [P, GPS + 1], F32, kind="ExternalOutput")

    with tile.TileContext(nc) as tc, ExitStack() as ctx:
        const = ctx.enter_context(tc.tile_pool(name="const", bufs=1))
        w1_sb = const.tile([P, ND, D_HID], MDT)
        for q4 in range(4):
            nq = ND // 4
            nc.sync.dma_start(
                out=w1_sb[:, q4 * nq : (q4 + 1) * nq, :],
                in_=w1p[:, q4 * nq : (q4 + 1) * nq, :],
            )
        b1_sb = const.tile([P, NH2], F32)
        nc.sync.dma_start(out=b1_sb[:], in_=b1p[:])
        qv_sb = const.tile([P, NH2, 2 * N_HEADS], MDT)
        nc.sync.dma_start(out=qv_sb[:], in_=qvp[:])

        # Pass-B layout, split per column c: partition s*16+h, free dim =
        # token c*G+j of subchunk s.  Separate tiles keep the dependency
        # tracking column-granular so pass-B slabs start mid-stream.
        bp = ctx.enter_context(tc.tile_pool(name="bp", bufs=1))
        RLc = [bp.tile([P, G], F32, name=f"RLc{c}") for c in range(GPS)]
        RVc = [bp.tile([P, G], F32, name=f"RVc{c}") for c in range(GPS)]
        RLH = bp.tile([P, HP], F32)
        RVH = bp.tile([P, HP], F32)

        xpool = ctx.enter_context(tc.tile_pool(name="xpool", bufs=5))
        ypool = ctx.enter_context(tc.tile_pool(name="ypool", bufs=6))
        rpool = ctx.enter_context(tc.tile_pool(name="rpool", bufs=4))
        spool = ctx.enter_context(tc.tile_pool(name="spool", bufs=2))
        psy = ctx.enter_context(tc.tile_pool(name="psy", bufs=4, space="PSUM"))
        pslv = ctx.enter_context(tc.tile_pool(name="pslv", bufs=3, space="PSUM"))

        # ---------------- pass A: MLP + probes ----------------
        def emit_mlp(xsl, gw, ytile, hh):
            """One hidden half: 8 DoubleRow (or 16 fp16) matmuls + relu.
            xsl(a, b) -> the [128, b-a, gw] slice of this group's x tile."""
            ypt = psy.tile([P, gw], F32, tag="ypsum")
            if DR is not None:
                for d2 in range(ND2):
                    nc.tensor.matmul(
                        ypt[:],
                        w1_sb[:, 2 * d2 : 2 * d2 + 2, hh * P : (hh + 1) * P],
                        xsl(2 * d2, 2 * d2 + 2),
                        start=(d2 == 0),
                        stop=(d2 == ND2 - 1),
                        perf_mode=DR,
                    )
            else:
                for d in range(ND):
                    nc.tensor.matmul(
                        ypt[:],
                        w1_sb[:, d, hh * P : (hh + 1) * P],
                        xsl(d, d + 1),
                        start=(d == 0),
                        stop=(d == ND - 1),
                    )
            nc.scalar.activation(
                ytile[:, hh, :], ypt[:], AF.Relu,
                bias=b1_sb[:, hh : hh + 1], scale=1.0 / w1_scale,
            )

        def emit_probe(g, gw, ytile):
            lv = pslv.tile([2 * N_HEADS, gw], F32, tag="lvp")
            if DR is not None:
                nc.tensor.matmul(
                    lv[:], qv_sb[:, 0:NH2, :], ytile[:, 0:NH2, :],
                    start=True, stop=True, perf_mode=DR,
                )
            else:
                for hh in range(NH2):
                    nc.tensor.matmul(
                        lv[:], qv_sb[:, hh, :], ytile[:, hh, :],
                        start=(hh == 0), stop=(hh == NH2 - 1),
                    )
            rl = rpool.tile([2 * N_HEADS, gw], F32, tag="relay")
            nc.vector.tensor_copy(out=rl[:], in_=lv[:])
            if HALO and g == NG - 1:
                # trailing halo-only group: subchunk NSUB-1's halo tokens
                h0 = (NSUB - 1) * N_HEADS
                nc.sync.dma_start(
                    out=RLH[h0 : h0 + N_HEADS, 0 : w - 1],
                    in_=rl[0:N_HEADS, 0 : w - 1],
                )
                nc.scalar.dma_start(
                    out=RVH[h0 : h0 + N_HEADS, 0 : w - 1],
                    in_=rl[N_HEADS : 2 * N_HEADS, 0 : w - 1],
                )
                return
            s, c = g // GPS, g % GPS
            rlo = s * N_HEADS
            nc.sync.dma_start(
                out=RLc[c][rlo : rlo + N_HEADS, 0:gw], in_=rl[0:N_HEADS, :]
            )
            nc.scalar.dma_start(
                out=RVc[c][rlo : rlo + N_HEADS, 0:gw],
                in_=rl[N_HEADS : 2 * N_HEADS, :],
            )
            if c == 0 and s > 0 and w > 1:
                h0 = (s - 1) * N_HEADS
                nc.sync.dma_start(
                    out=RLH[h0 : h0 + N_HEADS, 0 : w - 1],
                    in_=rl[0:N_HEADS, 0 : w - 1],
                )
                nc.scalar.dma_start(
                    out=RVH[h0 : h0 + N_HEADS, 0 : w - 1],
                    in_=rl[N_HEADS : 2 * N_HEADS, 0 : w - 1],
                )

        # Column-major pair order: all subchunks' column c before column
        # c+1, so pass-B slab t unblocks once column t+1 lands; the tiny
        # halo group runs first (it is independent and warms up the PE).
        pairs = [
            (2 * i * GPS + c, (2 * i + 1) * GPS + c)
            for c in COL_ORDER
            for i in range(NSUB // 2)
        ]
        pending = []
        if HALO:
            xt = xpool.tile([P, ND, LW], MDT, tag="xh")
            nc.sync.dma_start(out=xt[:], in_=xh[:])
            yt = ypool.tile([P, NH2, LW], MDT, tag="yt")
            for hh in range(NH2):
                emit_mlp(lambda a, b: xt[:, a:b, :], LW, yt, hh)
            pending.append((NG - 1, LW, yt))
        for j, pr in enumerate(pairs):
            xt = xpool.tile([P, 2, ND, G], MDT, tag="xt")
            if j <= 1:
                # Split the first paired load 4 ways so the PE fills sooner.
                nq = ND // 4
                for q4 in range(4):
                    nc.sync.dma_start(
                        out=xt[:, :, q4 * nq : (q4 + 1) * nq, :],
                        in_=xp[j, :, :, q4 * nq : (q4 + 1) * nq, :],
                    )
            else:
                # one chain per group: group A's MLP unblocks on its own 1
                # MiB chain, and two chains sustain a higher aggregate rate
                for g2 in range(2):
                    nc.sync.dma_start(
                        out=xt[:, g2, :, :], in_=xp[j, :, g2, :, :]
                    )
            for g2, g in enumerate(pr):
                yt = ypool.tile([P, NH2, G], MDT, tag="yt")
                for hh in range(NH2):
                    emit_mlp(
                        lambda a, b, g2=g2, xt=xt: xt[:, g2, a:b, :], G, yt, hh
                    )
                pending.append((g, G, yt))
            limit = 0 if (j + 1) % (NSUB // 2) == 0 else 4
            while len(pending) > limit:
                emit_probe(*pending.pop(0))
        for pnd in pending:
            emit_probe(*pnd)

        # ---------------- pass B: windowed softmax-mean scores ----------------
        # exp directly (no max shift: the shift cancels exactly within each
        # window and logits are O(1), far from the f32 exp overflow bound);
        # sliding-window sums via the DVE prefix scan: Z[j] = cs[j+w] - cs[j].
        # Slab t reads column tile t plus the first w-1 tokens of column
        # t+1 (or the halo tile), so it overlaps the tail of pass A.
        ADD = mybir.AluOpType.add
        BYP = mybir.AluOpType.bypass
        smax = bp.tile([P, GPS + 1], F32)
        for t in range(GPS):
            L = G + w - 1              # tokens this slab reads
            LP = (L + 16 + 15) // 16 * 16
            extL = RLc[t + 1] if t < GPS - 1 else RLH
            extV = RVc[t + 1] if t < GPS - 1 else RVH
            E = spool.tile([P, LP], F32, tag="E")
            EV = spool.tile([P, LP], F32, tag="EV")
            csZ = spool.tile([P, LP], F32, tag="csZ")
            csW = spool.tile([P, LP], F32, tag="csW")
            # main part: needs only column t.  Z chain on DVE, W chain on
            # GPSIMD so the two prefix scans run concurrently.
            nc.scalar.activation(
                E[:, 0:G], RLc[t][:, :], AF.Exp, scale=1.0 / qv_scale
            )
            nc.vector.tensor_mul(EV[:, 0:G], E[:, 0:G], RVc[t][:, :])
            nc.vector.memset(csZ[:, 0:1], 0.0)
            nc.vector.memset(csW[:, 0:1], 0.0)
            nc.vector.tensor_tensor_scan(
                out=csZ[:, 1 : 1 + G], data0=E[:, 0:G], data1=E[:, 0:G],
                initial=0.0, op0=ADD, op1=BYP,
            )
            nc.vector.tensor_tensor_scan(
                out=csW[:, 1 : 1 + G], data0=EV[:, 0:G], data1=EV[:, 0:G],
                initial=0.0, op0=ADD, op1=BYP,
            )
            if w > 1:
                # ext part: the first w-1 tokens of column t+1 (or the halo
                # tile), folded in late via scan chaining (initial=carry).
                nc.scalar.activation(
                    E[:, G:L], extL[:, 0 : w - 1], AF.Exp,
                    scale=1.0 / qv_scale,
                )
                nc.vector.tensor_mul(
                    EV[:, G:L], E[:, G:L], extV[:, 0 : w - 1]
                )
                nc.vector.tensor_tensor_scan(
                    out=csZ[:, 1 + G : 1 + L], data0=E[:, G:L],
                    data1=E[:, G:L], initial=csZ[:, G : G + 1],
                    op0=ADD, op1=BYP,
                )
                nc.vector.tensor_tensor_scan(
                    out=csW[:, 1 + G : 1 + L], data0=EV[:, G:L],
                    data1=EV[:, G:L], initial=csW[:, G : G + 1],
                    op0=ADD, op1=BYP,
                )
            # Z -> E, Wn -> EV (slab inputs are dead after the scans)
            nc.vector.tensor_sub(
                out=E[:, 0:G], in0=csZ[:, w : w + G], in1=csZ[:, 0:G]
            )
            nc.vector.tensor_sub(
                out=EV[:, 0:G], in0=csW[:, w : w + G], in1=csW[:, 0:G]
            )
            nc.vector.reciprocal_approx_fast(out=csZ[:, 0:G], in_=E[:, 0:G])
            nc.vector.tensor_mul(
                out=csW[:, 0:G], in0=EV[:, 0:G], in1=csZ[:, 0:G]
            )
            if t < GPS - 1:
                nc.vector.reduce_max(
                    out=smax[:, t : t + 1], in_=csW[:, 0:G], axis=AX.X
                )
            else:
                if SPLITL > 0:
                    nc.vector.reduce_max(
                        out=smax[:, t : t + 1], in_=csW[:, 0:SPLITL],
                        axis=AX.X,
                    )
                else:
                    nc.vector.memset(smax[:, t : t + 1], -3.0e38)
                if SPLITL < G:
                    nc.vector.reduce_max(
                        out=smax[:, t + 1 : t + 2],
                        in_=csW[:, SPLITL:G], axis=AX.X,
                    )
                else:
                    nc.vector.memset(smax[:, t + 1 : t + 2], -3.0e38)
        nc.sync.dma_start(out=res[:], in_=smax[:])

    nc.compile()
    return nc


MM_DTYPE = "f8"


def _get_nc(w: int):
    key = (w, MM_DTYPE)
    nc = _NC_CACHE.get(key)
    if nc is None:
        nc = _build(w, MM_DTYPE)
        _NC_CACHE[key] = nc
    return nc


def _mm_cast(a: np.ndarray) -> np.ndarray:
    """Convert to the matmul input dtype (host-side rounding)."""
    import ml_dtypes

    if MM_DTYPE == "f8":
        return a.astype(ml_dtypes.float8_e4m3)
    return a.astype(np.float16)


def _prep_inputs(x, w1, b1, queries, values, w):
    """Host-side packing: pad + scale + round + transpose into DMA-friendly
    layouts. Returns the per-core in_maps for run_bass_kernel_spmd."""
    NG = -(-(TPC + w - 1) // G)
    HALO = NG > GPC
    LW = min(G, ((w - 1 + 63) // 64) * 64) if HALO else G
    NGG = (N_CORES - 1) * GPC + NG  # distinct global groups incl. final halo
    w1_scale = W1_SCALE if MM_DTYPE == "f8" else 1.0
    qv_scale = QV_SCALE if MM_DTYPE == "f8" else 1.0
    xpad = np.zeros((NGG * G, D_MODEL), dtype=np.float32)
    xpad[:N_TOKENS] = x
    xr = _mm_cast(xpad)
    # [gg, p, d, t] = xpad[gg*G + t, d*128 + p]
    xg_all = np.ascontiguousarray(
        xr.reshape(NGG, G, ND, P).transpose(0, 3, 2, 1)
    )
    # Column-major pair permutation matching _build's processing order.
    perm = [
        g
        for c in COL_ORDER
        for i in range(NSUB // 2)
        for g in (2 * i * GPS + c, (2 * i + 1) * GPS + c)
    ]
    w1p = np.ascontiguousarray(
        _mm_cast(w1 * w1_scale).reshape(ND, P, D_HID).transpose(1, 0, 2)
    )
    b1p = np.ascontiguousarray(np.asarray(b1, np.float32).reshape(NH2, P).T)
    # Combined probe weights: [k, hh, m] with columns 0..15 = queries.T
    # chunk, columns 16..31 = values.T chunk.
    qv = np.concatenate(
        [np.asarray(queries, np.float32), np.asarray(values, np.float32)], axis=0
    )  # [32, 256]
    qvT = _mm_cast(qv * qv_scale).T.reshape(NH2, P, 2 * N_HEADS)  # [hh, k, m]
    qvp = np.ascontiguousarray(qvT.transpose(1, 0, 2))
    in_maps = []
    for c in range(N_CORES):
        body = xg_all[c * GPC : c * GPC + GPC][perm]  # [32, P, ND, G]
        m = {
            "xp": np.ascontiguousarray(
                body.reshape(NPAIR, 2, P, ND, G).transpose(0, 2, 1, 3, 4)
            ),
            "w1p": w1p,
            "b1p": b1p,
            "qvp": qvp,
        }
        if HALO:
            m["xh"] = np.ascontiguousarray(xg_all[c * GPC + NG - 1, :, :, 0:LW])
        in_maps.append(m)
    return in_maps


def _combine(results, w):
    """Host-side final reduction: per-core [128, GPS+1] -> scalar."""
    qv_scale = QV_SCALE if MM_DTYPE == "f8" else 1.0
    best = np.full(N_HEADS, -np.inf, dtype=np.float64)
    for c in range(N_CORES):
        r = np.asarray(results[c]["res"], dtype=np.float64).reshape(
            NSUB, N_HEADS, GPS + 1
        )
        if c == N_CORES - 1 and w >= 2:
            r = r.copy()
            r[NSUB - 1, :, GPS] = -np.inf  # windows past n - w on last core
        best = np.maximum(best, r.max(axis=(0, 2)))
    return np.asarray(best.sum() / qv_scale, dtype=np.float32)


def kernel(x, w1, b1, queries, values, window_size):
    from concourse.bass_utils import run_bass_kernel_spmd

    x = np.asarray(x, dtype=np.float32)
    w1 = np.asarray(w1, dtype=np.float32)
    b1 = np.asarray(b1, dtype=np.float32)
    queries = np.asarray(queries, dtype=np.float32)
    values = np.asarray(values, dtype=np.float32)
    w = int(np.asarray(window_size))
    assert x.shape == (N_TOKENS, D_MODEL), x.shape
    assert 1 <= w <= G + 1  # slab/halo duplication reads at most one column

    nc = _get_nc(w)
    in_maps = _prep_inputs(x, w1, b1, queries, values, w)
    last_err = None
    for attempt in range(3):
        try:
            out = run_bass_kernel_spmd(nc, in_maps, core_ids=list(range(N_CORES)))
            return _combine(out.results, w)
        except Exception as e:  # transient terminal/device failures
            last_err = e
            import time as _time

            _time.sleep(5.0 * (attempt + 1))
    raise last_err


# Optional: expose a traced run for profiling from test harnesses.
def kernel_traced(x, w1, b1, queries, values, window_size, tmpdir=None):
    from concourse.bass_utils import run_bass_kernel_spmd

    w = int(np.asarray(window_size))
    nc = _get_nc(w)
    in_maps = _prep_inputs(
        np.asarray(x, np.float32),
        np.asarray(w1, np.float32),
        np.asarray(b1, np.float32),
        np.asarray(queries, np.float32),
        np.asarray(values, np.float32),
        w,
    )
    out = run_bass_kernel_spmd(
        nc, in_maps, core_ids=list(range(N_CORES)), trace=True, tmpdir=tmpdir
    )
    return _combine(out.results, w), out


# revision 12
# speedup vs baseline: 1.2881x; 1.0978x over previous
"""TRN2 Bass kernel for nn_MaxRollingMeanAttentionProbe (sparse_attention).

Computation (reference):
    y      = relu(x @ w1 + b1)                    # [n, 256]
    logits = y @ queries.T ; vals = y @ values.T  # [n, 16]
    window i of size w: score_i = sum_j softmax(logits[i:i+w])_j * vals[i:i+w]_j
    out    = sum_h max_i score[i, h]              # scalar

Strategy: data-parallel over tokens across 8 NeuronCores with a recomputed
(w-1)-token halo, so no collectives are needed (the softmax shift cancels
exactly within any window).

Per core (one SPMD Tile program):
  pass A: stream host pre-transposed fp8e4 x in PAIRED 2 MiB DMAs (two
          512-token groups per trigger; bigger descriptor chains sustain
          higher HBM rate and halve the serial DMA-trigger load on the sync
          engine).  fp8 DoubleRow matmuls (2 k-tiles per instruction, 2x
          fp16 FLOP rate on HW) accumulate yT in fp32 PSUM; relu with
          scale=1/16 folds away the host-side x16 scaling of w1 (its 0.02
          magnitudes would otherwise sit in fp8e4's subnormal range and
          quantize at 2e-3 steps).  queries/values are similarly scaled x16
          and the scale is undone in the exp (logits) and on the host
          (vals).  A combined [queries; values] lhsT yields logits+vals
          stacked in one [32, 512] PSUM tile per group; probe work runs one
          PAIR behind the MLP.  Each probe result is copied PSUM->SBUF
          relay (DVE) and DMA'd into the per-column pass-B tiles from the
          GPSIMD software-DGE queue (DMA has no partition-base alignment
          restriction; engine ops need base % 32 == 0, and the sync engine
          is saturated issuing x loads).  Groups run in column-major order
          with the trailing halo group first, so each pass-B slab's column
          tiles close early.
  pass B: layout [128 partitions = 8 subchunks x 16 heads, tokens] split
          into GPS=4 per-column tiles + a (w-1)-wide halo tile, processed
          as 4 token slabs; slab t only depends on columns t and t+1, so
          its DVE/ScalarE work overlaps the tail of pass A instead of
          serializing after it.  Per slab: exp on ScalarE (scale=1/16);
          sliding-window sums via the DVE hardware prefix scan (fp32
          state) + shifted subtract; scores = Wsum/Z with a fast
          approximate reciprocal; max-reduce -> [128, 5].
Host: pack/scale/cast inputs (fp8e4 for matmul operands), final max/sum
      (tiny) including the /16 vals descale.
"""

import numpy as np

# Problem constants (shapes are fixed by the problem spec).
N_TOKENS = 131072
D_MODEL = 2048
D_HID = 256
N_HEADS = 16
N_CORES = 8
P = 128                    # SBUF partitions
G = 512                    # tokens per matmul group
TPC = N_TOKENS // N_CORES  # window starts per core (16384)
GPC = TPC // G             # groups per core without halo (32)
NSUB = 8                   # subchunks per core in pass B
SUB = TPC // NSUB          # window starts per subchunk (2048)
ND = D_MODEL // P          # 16 d_model chunks
ND2 = ND // 2              # 8 DoubleRow k-tile pairs
NH2 = D_HID // P           # 2 hidden halves
GPS = SUB // G             # groups (columns) per subchunk = pass-B slabs (4)
NPAIR = GPC // 2           # paired body-group loads (16)
COL_ORDER = (0, 1, 2, 3)   # halo tile (written by col-0 groups) lands early
W1_SCALE = 16.0            # host pre-scale on w1 (fp8 subnormal avoidance)
QV_SCALE = 16.0            # host pre-scale on queries/values

_NC_CACHE = {}


def _build(w: int, mmdt: str = "f8"):
    import concourse.bacc as bacc
    import concourse.tile as tile
    from concourse import mybir
    from contextlib import ExitStack

    F32 = mybir.dt.float32
    MDT = {"f8": mybir.dt.float8e4, "f16": mybir.dt.float16}[mmdt]
    DR = mybir.MatmulPerfMode.DoubleRow if mmdt == "f8" else None
    AF = mybir.ActivationFunctionType
    AX = mybir.AxisListType

    NG = -(-(TPC + w - 1) // G)    # groups per core incl. halo (33 for w>1)
    SUBLEN = SUB + w - 1           # tokens per subchunk
    SPLIT = SUB - w + 1            # j < SPLIT -> valid everywhere
    HALO = NG > GPC                # trailing halo-only group exists
    LW = min(G, ((w - 1 + 63) // 64) * 64) if HALO else G
    SPLITL = SPLIT - (GPS - 1) * G  # SPLIT within the last slab (G - w + 1)
    HP = ((w - 1 + 15) // 16 * 16) if w > 1 else 16  # halo tile width

    w1_scale = W1_SCALE if mmdt == "f8" else 1.0
    qv_scale = QV_SCALE if mmdt == "f8" else 1.0

    nc = bacc.Bacc(
        "TRN2",
        target_bir_lowering=False,
        debug=False,
        enable_asserts=False,
        num_devices=N_CORES,
    )
    # Paired body groups in column-major processing order:
    # xp[j, :, g2] = body group perm[2j + g2] (see _prep_inputs).
    xp = nc.dram_tensor("xp", [NPAIR, P, 2, ND, G], MDT, kind="ExternalInput")
    if HALO:
        xh = nc.dram_tensor("xh", [P, ND, LW], MDT, kind="ExternalInput")
    w1p = nc.dram_tensor("w1p", [P, ND, D_HID], MDT, kind="ExternalInput")
    b1p = nc.dram_tensor("b1p", [P, NH2], F32, kind="ExternalInput")
    # Combined probe weights [k, hh, m]: columns 0..15 = queries, 16..31 =
    # values -> one matmul yields logits/vals stacked in PSUM rows 0..31.
    qvp = nc.dram_tensor("qvp", [P, NH2, 2 * N_HEADS], MDT, kind="ExternalInput")
    res = nc.dram_tensor("res", [P, GPS + 1], F32, kind="ExternalOutput")

    with tile.TileContext(nc) as tc, ExitStack() as ctx:
        const = ctx.enter_context(tc.tile_pool(name="const", bufs=1))
        w1_sb = const.tile([P, ND, D_HID], MDT)
        for q4 in range(4):
            nq = ND // 4
            nc.sync.dma_start(
                out=w1_sb[:, q4 * nq : (q4 + 1) * nq, :],
                in_=w1p[:, q4 * nq : (q4 + 1) * nq, :],
            )
        b1_sb = const.tile([P, NH2], F32)
        nc.sync.dma_start(out=b1_sb[:], in_=b1p[:])
        qv_sb = const.tile([P, NH2, 2 * N_HEADS], MDT)
        nc.sync.dma_start(out=qv_sb[:], in_=qvp[:])

        # Pass-B layout, split per column c: partition s*16+h, free dim =
        # token c*G+j of subchunk s.  Separate tiles keep the dependency
        # tracking column-granular so pass-B slabs start mid-stream.
        bp = ctx.enter_context(tc.tile_pool(name="bp", bufs=1))
        RLc = [bp.tile([P, G], F32, name=f"RLc{c}") for c in range(GPS)]
        RVc = [bp.tile([P, G], F32, name=f"RVc{c}") for c in range(GPS)]
        RLH = bp.tile([P, HP], F32)
        RVH = bp.tile([P, HP], F32)

        xpool = ctx.enter_context(tc.tile_pool(name="xpool", bufs=4))
        ypool = ctx.enter_context(tc.tile_pool(name="ypool", bufs=6))
        rpool = ctx.enter_context(tc.tile_pool(name="rpool", bufs=4))
        spool = ctx.enter_context(tc.tile_pool(name="spool", bufs=2))
        psy = ctx.enter_context(tc.tile_pool(name="psy", bufs=4, space="PSUM"))
        pslv = ctx.enter_context(tc.tile_pool(name="pslv", bufs=3, space="PSUM"))

        # ---------------- pass A: MLP + probes ----------------
        def emit_mlp(xsl, gw, ytile, hh):
            """One hidden half: 8 DoubleRow (or 16 fp16) matmuls + relu.
            xsl(a, b) -> the [128, b-a, gw] slice of this group's x tile."""
            ypt = psy.tile([P, gw], F32, tag="ypsum")
            if DR is not None:
                for d2 in range(ND2):
                    nc.tensor.matmul(
                        ypt[:],
                        w1_sb[:, 2 * d2 : 2 * d2 + 2, hh * P : (hh + 1) * P],
                        xsl(2 * d2, 2 * d2 + 2),
                        start=(d2 == 0),
                        stop=(d2 == ND2 - 1),
                        perf_mode=DR,
                    )
            else:
                for d in range(ND):
                    nc.tensor.matmul(
                        ypt[:],
                        w1_sb[:, d, hh * P : (hh + 1) * P],
                        xsl(d, d + 1),
                        start=(d == 0),
                        stop=(d == ND - 1),
                    )
            nc.scalar.activation(
                ytile[:, hh, :], ypt[:], AF.Relu,
                bias=b1_sb[:, hh : hh + 1], scale=1.0 / w1_scale,
            )

        def emit_probe(g, gw, ytile):
            lv = pslv.tile([2 * N_HEADS, gw], F32, tag="lvp")
            if DR is not None:
                nc.tensor.matmul(
                    lv[:], qv_sb[:, 0:NH2, :], ytile[:, 0:NH2, :],
                    start=True, stop=True, perf_mode=DR,
                )
            else:
                for hh in range(NH2):
                    nc.tensor.matmul(
                        lv[:], qv_sb[:, hh, :], ytile[:, hh, :],
                        start=(hh == 0), stop=(hh == NH2 - 1),
                    )
            rl = rpool.tile([2 * N_HEADS, gw], F32, tag="relay")
            nc.vector.tensor_copy(out=rl[:], in_=lv[:])
            if HALO and g == NG - 1:
                # trailing halo-only group: subchunk NSUB-1's halo tokens
                h0 = (NSUB - 1) * N_HEADS
                nc.sync.dma_start(
                    out=RLH[h0 : h0 + N_HEADS, 0 : w - 1],
                    in_=rl[0:N_HEADS, 0 : w - 1],
                )
                nc.scalar.dma_start(
                    out=RVH[h0 : h0 + N_HEADS, 0 : w - 1],
                    in_=rl[N_HEADS : 2 * N_HEADS, 0 : w - 1],
                )
                return
            s, c = g // GPS, g % GPS
            rlo = s * N_HEADS
            nc.sync.dma_start(
                out=RLc[c][rlo : rlo + N_HEADS, 0:gw], in_=rl[0:N_HEADS, :]
            )
            nc.scalar.dma_start(
                out=RVc[c][rlo : rlo + N_HEADS, 0:gw],
                in_=rl[N_HEADS : 2 * N_HEADS, :],
            )
            if c == 0 and s > 0 and w > 1:
                h0 = (s - 1) * N_HEADS
                nc.sync.dma_start(
                    out=RLH[h0 : h0 + N_HEADS, 0 : w - 1],
                    in_=rl[0:N_HEADS, 0 : w - 1],
                )
                nc.scalar.dma_start(
                    out=RVH[h0 : h0 + N_HEADS, 0 : w - 1],
                    in_=rl[N_HEADS : 2 * N_HEADS, 0 : w - 1],
                )

        # Column-major pair order: all subchunks' column c before column
        # c+1, so pass-B slab t unblocks once column t+1 lands; the tiny
        # halo group runs first (it is independent and warms up the PE).
        pairs = [
            (2 * i * GPS + c, (2 * i + 1) * GPS + c)
            for c in COL_ORDER
            for i in range(NSUB // 2)
        ]
        pending = []
        if HALO:
            xt = xpool.tile([P, ND, LW], MDT, tag="xh")
            nc.sync.dma_start(out=xt[:], in_=xh[:])
            yt = ypool.tile([P, NH2, LW], MDT, tag="yt")
            for hh in range(NH2):
                emit_mlp(lambda a, b: xt[:, a:b, :], LW, yt, hh)
            pending.append((NG - 1, LW, yt))
        for j, pr in enumerate(pairs):
            xt = xpool.tile([P, 2, ND, G], MDT, tag="xt")
            if j <= 1:
                # Split the first paired load 4 ways so the PE fills sooner.
                nq = ND // 4
                for q4 in range(4):
                    nc.sync.dma_start(
                        out=xt[:, :, q4 * nq : (q4 + 1) * nq, :],
                        in_=xp[j, :, :, q4 * nq : (q4 + 1) * nq, :],
                    )
            else:
                nc.sync.dma_start(out=xt[:], in_=xp[j])
            for g2, g in enumerate(pr):
                yt = ypool.tile([P, NH2, G], MDT, tag="yt")
                for hh in range(NH2):
                    emit_mlp(
                        lambda a, b, g2=g2, xt=xt: xt[:, g2, a:b, :], G, yt, hh
                    )
                pending.append((g, G, yt))
            limit = 0 if (j + 1) % (NSUB // 2) == 0 else 4
            while len(pending) > limit:
                emit_probe(*pending.pop(0))
        for pnd in pending:
            emit_probe(*pnd)

        # ---------------- pass B: windowed softmax-mean scores ----------------
        # exp directly (no max shift: the shift cancels exactly within each
        # window and logits are O(1), far from the f32 exp overflow bound);
        # sliding-window sums via the DVE prefix scan: Z[j] = cs[j+w] - cs[j].
        # Slab t reads column tile t plus the first w-1 tokens of column
        # t+1 (or the halo tile), so it overlaps the tail of pass A.
        ADD = mybir.AluOpType.add
        BYP = mybir.AluOpType.bypass
        smax = bp.tile([P, GPS + 1], F32)
        for t in range(GPS):
            L = G + w - 1              # tokens this slab reads
            LP = (L + 16 + 15) // 16 * 16
            extL = RLc[t + 1] if t < GPS - 1 else RLH
            extV = RVc[t + 1] if t < GPS - 1 else RVH
            E = spool.tile([P, LP], F32, tag="E")
            EV = spool.tile([P, LP], F32, tag="EV")
            csZ = spool.tile([P, LP], F32, tag="csZ")
            csW = spool.tile([P, LP], F32, tag="csW")
            # main part: needs only column t.  Z chain on DVE, W chain on
            # GPSIMD so the two prefix scans run concurrently.
            nc.scalar.activation(
                E[:, 0:G], RLc[t][:, :], AF.Exp, scale=1.0 / qv_scale
            )
            nc.vector.tensor_mul(EV[:, 0:G], E[:, 0:G], RVc[t][:, :])
            nc.vector.memset(csZ[:, 0:1], 0.0)
            nc.vector.memset(csW[:, 0:1], 0.0)
            nc.vector.tensor_tensor_scan(
                out=csZ[:, 1 : 1 + G], data0=E[:, 0:G], data1=E[:, 0:G],
                initial=0.0, op0=ADD, op1=BYP,
            )
            nc.vector.tensor_tensor_scan(
                out=csW[:, 1 : 1 + G], data0=EV[:, 0:G], data1=EV[:, 0:G],
                initial=0.0, op0=ADD, op1=BYP,
            )
            if w > 1:
                # ext part: the first w-1 tokens of column t+1 (or the halo
                # tile), folded in late via scan chaining (initial=carry).
                nc.scalar.activation(
                    E[:, G:L], extL[:, 0 : w - 1], AF.Exp,
                    scale=1.0 / qv_scale,
                )
                nc.vector.tensor_mul(
                    EV[:, G:L], E[:, G:L], extV[:, 0 : w - 1]
                )
                nc.vector.tensor_tensor_scan(
                    out=csZ[:, 1 + G : 1 + L], data0=E[:, G:L],
                    data1=E[:, G:L], initial=csZ[:, G : G + 1],
                    op0=ADD, op1=BYP,
                )
                nc.vector.tensor_tensor_scan(
                    out=csW[:, 1 + G : 1 + L], data0=EV[:, G:L],
                    data1=EV[:, G:L], initial=csW[:, G : G + 1],
                    op0=ADD, op1=BYP,
                )
            # Z -> E, Wn -> EV (slab inputs are dead after the scans)
            nc.vector.tensor_sub(
                out=E[:, 0:G], in0=csZ[:, w : w + G], in1=csZ[:, 0:G]
            )
            nc.vector.tensor_sub(
                out=EV[:, 0:G], in0=csW[:, w : w + G], in1=csW[:, 0:G]
            )
            nc.vector.reciprocal_approx_fast(out=csZ[:, 0:G], in_=E[:, 0:G])
            nc.vector.tensor_mul(
                out=csW[:, 0:G], in0=EV[:, 0:G], in1=csZ[:, 0:G]
            )
            if t < GPS - 1:
                nc.vector.reduce_max(
                    out=smax[:, t : t + 1], in_=csW[:, 0:G], axis=AX.X
                )
            else:
                if SPLITL > 0:
                    nc.vector.reduce_max(
                        out=smax[:, t : t + 1], in_=csW[:, 0:SPLITL],
                        axis=AX.X,
                    )
                else:
                    nc.vector.memset(smax[:, t : t + 1], -3.0e38)
                if SPLITL < G:
                    nc.vector.reduce_max(
                        out=smax[:, t + 1 : t + 2],
                        in_=csW[:, SPLITL:G], axis=AX.X,
                    )
                else:
                    nc.vector.memset(smax[:, t + 1 : t + 2], -3.0e38)
        nc.sync.dma_start(out=res[:], in_=smax[:])

    nc.compile()
    return nc


MM_DTYPE = "f8"


def _get_nc(w: int):
    key = (w, MM_DTYPE)
    nc = _NC_CACHE.get(key)
    if nc is None:
        nc = _build(w, MM_DTYPE)
        _NC_CACHE[key] = nc
    return nc


def _mm_cast(a: np.ndarray) -> np.ndarray:
    """Convert to the matmul input dtype (host-side rounding)."""
    import ml_dtypes

    if MM_DTYPE == "f8":
        return a.astype(ml_dtypes.float8_e4m3)
    return a.astype(np.float16)


def _prep_inputs(x, w1, b1, queries, values, w):
    """Host-side packing: pad + scale + round + transpose into DMA-friendly
    layouts. Returns the per-core in_maps for run_bass_kernel_spmd."""
    NG = -(-(TPC + w - 1) // G)
    HALO = NG > GPC
    LW = min(G, ((w - 1 + 63) // 64) * 64) if HALO else G
    NGG = (N_CORES - 1) * GPC + NG  # distinct global groups incl. final halo
    w1_scale = W1_SCALE if MM_DTYPE == "f8" else 1.0
    qv_scale = QV_SCALE if MM_DTYPE == "f8" else 1.0
    xpad = np.zeros((NGG * G, D_MODEL), dtype=np.float32)
    xpad[:N_TOKENS] = x
    xr = _mm_cast(xpad)
    # [gg, p, d, t] = xpad[gg*G + t, d*128 + p]
    xg_all = np.ascontiguousarray(
        xr.reshape(NGG, G, ND, P).transpose(0, 3, 2, 1)
    )
    # Column-major pair permutation matching _build's processing order.
    perm = [
        g
        for c in COL_ORDER
        for i in range(NSUB // 2)
        for g in (2 * i * GPS + c, (2 * i + 1) * GPS + c)
    ]
    w1p = np.ascontiguousarray(
        _mm_cast(w1 * w1_scale).reshape(ND, P, D_HID).transpose(1, 0, 2)
    )
    b1p = np.ascontiguousarray(np.asarray(b1, np.float32).reshape(NH2, P).T)
    # Combined probe weights: [k, hh, m] with columns 0..15 = queries.T
    # chunk, columns 16..31 = values.T chunk.
    qv = np.concatenate(
        [np.asarray(queries, np.float32), np.asarray(values, np.float32)], axis=0
    )  # [32, 256]
    qvT = _mm_cast(qv * qv_scale).T.reshape(NH2, P, 2 * N_HEADS)  # [hh, k, m]
    qvp = np.ascontiguousarray(qvT.transpose(1, 0, 2))
    in_maps = []
    for c in range(N_CORES):
        body = xg_all[c * GPC : c * GPC + GPC][perm]  # [32, P, ND, G]
        m = {
            "xp": np.ascontiguousarray(
                body.reshape(NPAIR, 2, P, ND, G).transpose(0, 2, 1, 3, 4)
            ),
            "w1p": w1p,
            "b1p": b1p,
            "qvp": qvp,
        }
        if HALO:
            m["xh"] = np.ascontiguousarray(xg_all[c * GPC + NG - 1, :, :, 0:LW])
        in_maps.append(m)
    return in_maps


def _combine(results, w):
    """Host-side final reduction: per-core [128, GPS+1] -> scalar."""
    qv_scale = QV_SCALE if MM_DTYPE == "f8" else 1.0
    best = np.full(N_HEADS, -np.inf, dtype=np.float64)
    for c in range(N_CORES):
        r = np.asarray(results[c]["res"], dtype=np.float64).reshape(
            NSUB, N_HEADS, GPS + 1
        )
        if c == N_CORES - 1 and w >= 2:
            r = r.copy()
            r[NSUB - 1, :, GPS] = -np.inf  # windows past n - w on last core
        best = np.maximum(best, r.max(axis=(0, 2)))
    return np.asarray(best.sum() / qv_scale, dtype=np.float32)


def kernel(x, w1, b1, queries, values, window_size):
    from concourse.bass_utils import run_bass_kernel_spmd

    x = np.asarray(x, dtype=np.float32)
    w1 = np.asarray(w1, dtype=np.float32)
    b1 = np.asarray(b1, dtype=np.float32)
    queries = np.asarray(queries, dtype=np.float32)
    values = np.asarray(values, dtype=np.float32)
    w = int(np.asarray(window_size))
    assert x.shape == (N_TOKENS, D_MODEL), x.shape
    assert 1 <= w <= G + 1  # slab/halo duplication reads at most one column

    nc = _get_nc(w)
    in_maps = _prep_inputs(x, w1, b1, queries, values, w)
    last_err = None
    for attempt in range(3):
        try:
            out = run_bass_kernel_spmd(nc, in_maps, core_ids=list(range(N_CORES)))
            return _combine(out.results, w)
        except Exception as e:  # transient terminal/device failures
            last_err = e
            import time as _time

            _time.sleep(5.0 * (attempt + 1))
    raise last_err


# Optional: expose a traced run for profiling from test harnesses.
def kernel_traced(x, w1, b1, queries, values, window_size, tmpdir=None):
    from concourse.bass_utils import run_bass_kernel_spmd

    w = int(np.asarray(window_size))
    nc = _get_nc(w)
    in_maps = _prep_inputs(
        np.asarray(x, np.float32),
        np.asarray(w1, np.float32),
        np.asarray(b1, np.float32),
        np.asarray(queries, np.float32),
        np.asarray(values, np.float32),
        w,
    )
    out = run_bass_kernel_spmd(
        nc, in_maps, core_ids=list(range(N_CORES)), trace=True, tmpdir=tmpdir
    )
    return _combine(out.results, w), out


# revision 13
# speedup vs baseline: 1.3012x; 1.0102x over previous
"""TRN2 Bass kernel for nn_MaxRollingMeanAttentionProbe (sparse_attention).

Computation (reference):
    y      = relu(x @ w1 + b1)                    # [n, 256]
    logits = y @ queries.T ; vals = y @ values.T  # [n, 16]
    window i of size w: score_i = sum_j softmax(logits[i:i+w])_j * vals[i:i+w]_j
    out    = sum_h max_i score[i, h]              # scalar

Strategy: data-parallel over tokens across 8 NeuronCores with a recomputed
(w-1)-token halo, so no collectives are needed (the softmax shift cancels
exactly within any window).

Per core (one SPMD Tile program):
  pass A: stream host pre-transposed fp8e4 x in PAIRED 2 MiB DMAs (two
          512-token groups per trigger; bigger descriptor chains sustain
          higher HBM rate and halve the serial DMA-trigger load on the sync
          engine).  fp8 DoubleRow matmuls (2 k-tiles per instruction, 2x
          fp16 FLOP rate on HW) accumulate yT in fp32 PSUM; relu with
          scale=1/16 folds away the host-side x16 scaling of w1 (its 0.02
          magnitudes would otherwise sit in fp8e4's subnormal range and
          quantize at 2e-3 steps).  queries/values are similarly scaled x16
          and the scale is undone in the exp (logits) and on the host
          (vals).  A combined [queries; values] lhsT yields logits+vals
          stacked in one [32, 512] PSUM tile per group; probe work runs one
          PAIR behind the MLP.  Each probe result is copied PSUM->SBUF
          relay (DVE) and DMA'd into the per-column pass-B tiles from the
          GPSIMD software-DGE queue (DMA has no partition-base alignment
          restriction; engine ops need base % 32 == 0, and the sync engine
          is saturated issuing x loads).  Groups run in column-major order
          with the trailing halo group first, so each pass-B slab's column
          tiles close early.
  pass B: layout [128 partitions = 8 subchunks x 16 heads, tokens] split
          into GPS=4 per-column tiles + a (w-1)-wide halo tile, processed
          as 4 token slabs; slab t only depends on columns t and t+1, so
          its DVE/ScalarE work overlaps the tail of pass A instead of
          serializing after it.  Per slab: exp on ScalarE (scale=1/16);
          sliding-window sums via the DVE hardware prefix scan (fp32
          state) + shifted subtract; scores = Wsum/Z with a fast
          approximate reciprocal; max-reduce -> [128, 5].
Host: pack/scale/cast inputs (fp8e4 for matmul operands), final max/sum
      (tiny) including the /16 vals descale.
"""

import numpy as np

# Problem constants (shapes are fixed by the problem spec).
N_TOKENS = 131072
D_MODEL = 2048
D_HID = 256
N_HEADS = 16
N_CORES = 8
P = 128                    # SBUF partitions
G = 512                    # tokens per matmul group
TPC = N_TOKENS // N_CORES  # window starts per core (16384)
GPC = TPC // G             # groups per core without halo (32)
NSUB = 8                   # subchunks per core in pass B
SUB = TPC // NSUB          # window starts per subchunk (2048)
ND = D_MODEL // P          # 16 d_model chunks
ND2 = ND // 2              # 8 DoubleRow k-tile pairs
NH2 = D_HID // P           # 2 hidden halves
GPS = SUB // G             # groups (columns) per subchunk = pass-B slabs (4)
NPAIR = GPC // 2           # paired body-group loads (16)
COL_ORDER = (0, 1, 2, 3)   # halo tile (written by col-0 groups) lands early
W1_SCALE = 16.0            # host pre-scale on w1 (fp8 subnormal avoidance)
QV_SCALE = 16.0            # host pre-scale on queries/values

_NC_CACHE = {}


def _build(w: int, mmdt: str = "f8"):
    import concourse.bacc as bacc
    import concourse.tile as tile
    from concourse import mybir
    from contextlib import ExitStack

    F32 = mybir.dt.float32
    MDT = {"f8": mybir.dt.float8e4, "f16": mybir.dt.float16}[mmdt]
    DR = mybir.MatmulPerfMode.DoubleRow if mmdt == "f8" else None
    AF = mybir.ActivationFunctionType
    AX = mybir.AxisListType

    NG = -(-(TPC + w - 1) // G)    # groups per core incl. halo (33 for w>1)
    SUBLEN = SUB + w - 1           # tokens per subchunk
    SPLIT = SUB - w + 1            # j < SPLIT -> valid everywhere
    HALO = NG > GPC                # trailing halo-only group exists
    LW = min(G, ((w - 1 + 63) // 64) * 64) if HALO else G
    SPLITL = SPLIT - (GPS - 1) * G  # SPLIT within the last slab (G - w + 1)
    HP = ((w - 1 + 15) // 16 * 16) if w > 1 else 16  # halo tile width

    w1_scale = W1_SCALE if mmdt == "f8" else 1.0
    qv_scale = QV_SCALE if mmdt == "f8" else 1.0

    nc = bacc.Bacc(
        "TRN2",
        target_bir_lowering=False,
        debug=False,
        enable_asserts=False,
        num_devices=N_CORES,
    )
    # Paired body groups in column-major processing order:
    # xp[j, :, g2] = body group perm[2j + g2] (see _prep_inputs).
    xp = nc.dram_tensor("xp", [NPAIR, P, 2, ND, G], MDT, kind="ExternalInput")
    if HALO:
        xh = nc.dram_tensor("xh", [P, ND, LW], MDT, kind="ExternalInput")
    w1p = nc.dram_tensor("w1p", [P, ND, D_HID], MDT, kind="ExternalInput")
    b1p = nc.dram_tensor("b1p", [P, NH2], F32, kind="ExternalInput")
    # Combined probe weights [k, hh, m]: columns 0..15 = queries, 16..31 =
    # values -> one matmul yields logits/vals stacked in PSUM rows 0..31.
    qvp = nc.dram_tensor("qvp", [P, NH2, 2 * N_HEADS], MDT, kind="ExternalInput")
    res = nc.dram_tensor("res", [P, GPS + 1], F32, kind="ExternalOutput")

    with tile.TileContext(nc) as tc, ExitStack() as ctx:
        const = ctx.enter_context(tc.tile_pool(name="const", bufs=1))
        w1_sb = const.tile([P, ND, D_HID], MDT)
        for q4 in range(4):
            nq = ND // 4
            nc.sync.dma_start(
                out=w1_sb[:, q4 * nq : (q4 + 1) * nq, :],
                in_=w1p[:, q4 * nq : (q4 + 1) * nq, :],
            )
        b1_sb = const.tile([P, NH2], F32)
        nc.sync.dma_start(out=b1_sb[:], in_=b1p[:])
        qv_sb = const.tile([P, NH2, 2 * N_HEADS], MDT)
        nc.sync.dma_start(out=qv_sb[:], in_=qvp[:])

        # Pass-B layout, split per column c: partition s*16+h, free dim =
        # token c*G+j of subchunk s.  Separate tiles keep the dependency
        # tracking column-granular so pass-B slabs start mid-stream.
        bp = ctx.enter_context(tc.tile_pool(name="bp", bufs=1))
        RLc = [bp.tile([P, G], F32, name=f"RLc{c}") for c in range(GPS)]
        RVc = [bp.tile([P, G], F32, name=f"RVc{c}") for c in range(GPS)]
        RLH = bp.tile([P, HP], F32)
        RVH = bp.tile([P, HP], F32)

        xpool = ctx.enter_context(tc.tile_pool(name="xpool", bufs=4))
        ypool = ctx.enter_context(tc.tile_pool(name="ypool", bufs=6))
        rpool = ctx.enter_context(tc.tile_pool(name="rpool", bufs=4))
        spool = ctx.enter_context(tc.tile_pool(name="spool", bufs=2))
        psy = ctx.enter_context(tc.tile_pool(name="psy", bufs=4, space="PSUM"))
        pslv = ctx.enter_context(tc.tile_pool(name="pslv", bufs=3, space="PSUM"))

        # ---------------- pass A: MLP + probes ----------------
        def emit_mlp(xsl, gw, ytile, hh):
            """One hidden half: 8 DoubleRow (or 16 fp16) matmuls + relu.
            xsl(a, b) -> the [128, b-a, gw] slice of this group's x tile."""
            ypt = psy.tile([P, gw], F32, tag="ypsum")
            if DR is not None:
                for d2 in range(ND2):
                    nc.tensor.matmul(
                        ypt[:],
                        w1_sb[:, 2 * d2 : 2 * d2 + 2, hh * P : (hh + 1) * P],
                        xsl(2 * d2, 2 * d2 + 2),
                        start=(d2 == 0),
                        stop=(d2 == ND2 - 1),
                        perf_mode=DR,
                    )
            else:
                for d in range(ND):
                    nc.tensor.matmul(
                        ypt[:],
                        w1_sb[:, d, hh * P : (hh + 1) * P],
                        xsl(d, d + 1),
                        start=(d == 0),
                        stop=(d == ND - 1),
                    )
            nc.scalar.activation(
                ytile[:, hh, :], ypt[:], AF.Relu,
                bias=b1_sb[:, hh : hh + 1], scale=1.0 / w1_scale,
            )

        def emit_probe(g, gw, ytile):
            lv = pslv.tile([2 * N_HEADS, gw], F32, tag="lvp")
            if DR is not None:
                nc.tensor.matmul(
                    lv[:], qv_sb[:, 0:NH2, :], ytile[:, 0:NH2, :],
                    start=True, stop=True, perf_mode=DR,
                )
            else:
                for hh in range(NH2):
                    nc.tensor.matmul(
                        lv[:], qv_sb[:, hh, :], ytile[:, hh, :],
                        start=(hh == 0), stop=(hh == NH2 - 1),
                    )
            rl = rpool.tile([2 * N_HEADS, gw], F32, tag="relay")
            nc.vector.tensor_copy(out=rl[:], in_=lv[:])
            if HALO and g == NG - 1:
                # trailing halo-only group: subchunk NSUB-1's halo tokens
                h0 = (NSUB - 1) * N_HEADS
                nc.sync.dma_start(
                    out=RLH[h0 : h0 + N_HEADS, 0 : w - 1],
                    in_=rl[0:N_HEADS, 0 : w - 1],
                )
                nc.scalar.dma_start(
                    out=RVH[h0 : h0 + N_HEADS, 0 : w - 1],
                    in_=rl[N_HEADS : 2 * N_HEADS, 0 : w - 1],
                )
                return
            s, c = g // GPS, g % GPS
            rlo = s * N_HEADS
            nc.sync.dma_start(
                out=RLc[c][rlo : rlo + N_HEADS, 0:gw], in_=rl[0:N_HEADS, :]
            )
            nc.scalar.dma_start(
                out=RVc[c][rlo : rlo + N_HEADS, 0:gw],
                in_=rl[N_HEADS : 2 * N_HEADS, :],
            )
            if c == 0 and s > 0 and w > 1:
                h0 = (s - 1) * N_HEADS
                nc.sync.dma_start(
                    out=RLH[h0 : h0 + N_HEADS, 0 : w - 1],
                    in_=rl[0:N_HEADS, 0 : w - 1],
                )
                nc.scalar.dma_start(
                    out=RVH[h0 : h0 + N_HEADS, 0 : w - 1],
                    in_=rl[N_HEADS : 2 * N_HEADS, 0 : w - 1],
                )

        # Column-major pair order: all subchunks' column c before column
        # c+1, so pass-B slab t unblocks once column t+1 lands; the tiny
        # halo group runs first (it is independent and warms up the PE).
        pairs = [
            (2 * i * GPS + c, (2 * i + 1) * GPS + c)
            for c in COL_ORDER
            for i in range(NSUB // 2)
        ]
        pending = []
        if HALO:
            xt = xpool.tile([P, ND, LW], MDT, tag="xh")
            nc.sync.dma_start(out=xt[:], in_=xh[:])
            yt = ypool.tile([P, NH2, LW], MDT, tag="yt")
            for hh in range(NH2):
                emit_mlp(lambda a, b: xt[:, a:b, :], LW, yt, hh)
            pending.append((NG - 1, LW, yt))
        for j, pr in enumerate(pairs):
            xt = xpool.tile([P, 2, ND, G], MDT, tag="xt")
            if j <= 1:
                # Split the first paired load 4 ways so the PE fills sooner.
                nq = ND // 4
                for q4 in range(4):
                    nc.sync.dma_start(
                        out=xt[:, :, q4 * nq : (q4 + 1) * nq, :],
                        in_=xp[j, :, :, q4 * nq : (q4 + 1) * nq, :],
                    )
            else:
                nc.sync.dma_start(out=xt[:], in_=xp[j])
            for g2, g in enumerate(pr):
                yt = ypool.tile([P, NH2, G], MDT, tag="yt")
                for hh in range(NH2):
                    emit_mlp(
                        lambda a, b, g2=g2, xt=xt: xt[:, g2, a:b, :], G, yt, hh
                    )
                pending.append((g, G, yt))
            limit = 0 if (j + 1) % (NSUB // 2) == 0 else 4
            while len(pending) > limit:
                emit_probe(*pending.pop(0))
        for pnd in pending:
            emit_probe(*pnd)

        # ---------------- pass B: windowed softmax-mean scores ----------------
        # exp directly (no max shift: the shift cancels exactly within each
        # window and logits are O(1), far from the f32 exp overflow bound);
        # sliding-window sums via the DVE prefix scan: Z[j] = cs[j+w] - cs[j].
        # Slab t reads column tile t plus the first w-1 tokens of column
        # t+1 (or the halo tile), so it overlaps the tail of pass A.
        ADD = mybir.AluOpType.add
        BYP = mybir.AluOpType.bypass
        smax = bp.tile([P, GPS + 1], F32)
        for t in range(GPS):
            L = G + w - 1              # tokens this slab reads
            LP = (L + 16 + 15) // 16 * 16
            extL = RLc[t + 1] if t < GPS - 1 else RLH
            extV = RVc[t + 1] if t < GPS - 1 else RVH
            E = spool.tile([P, LP], F32, tag="E")
            nc.scalar.activation(
                E[:, 0:G], RLc[t][:, :], AF.Exp, scale=1.0 / qv_scale
            )
            EV = spool.tile([P, LP], F32, tag="EV")
            nc.vector.tensor_mul(EV[:, 0:G], E[:, 0:G], RVc[t][:, :])
            if w > 1:
                nc.scalar.activation(
                    E[:, G:L], extL[:, 0 : w - 1], AF.Exp,
                    scale=1.0 / qv_scale,
                )
                nc.vector.tensor_mul(
                    EV[:, G:L], E[:, G:L], extV[:, 0 : w - 1]
                )
            csZ = spool.tile([P, LP], F32, tag="csZ")
            csW = spool.tile([P, LP], F32, tag="csW")
            nc.vector.memset(csZ[:, 0:1], 0.0)
            nc.vector.memset(csW[:, 0:1], 0.0)
            nc.vector.tensor_tensor_scan(
                out=csZ[:, 1 : 1 + L], data0=E[:, 0:L], data1=E[:, 0:L],
                initial=0.0, op0=ADD, op1=BYP,
            )
            nc.vector.tensor_tensor_scan(
                out=csW[:, 1 : 1 + L], data0=EV[:, 0:L], data1=EV[:, 0:L],
                initial=0.0, op0=ADD, op1=BYP,
            )
            # Z -> E, Wn -> EV (slab inputs are dead after the scans)
            nc.vector.tensor_sub(
                out=E[:, 0:G], in0=csZ[:, w : w + G], in1=csZ[:, 0:G]
            )
            nc.vector.tensor_sub(
                out=EV[:, 0:G], in0=csW[:, w : w + G], in1=csW[:, 0:G]
            )
            nc.vector.reciprocal_approx_fast(out=csZ[:, 0:G], in_=E[:, 0:G])
            nc.vector.tensor_mul(
                out=csW[:, 0:G], in0=EV[:, 0:G], in1=csZ[:, 0:G]
            )
            if t < GPS - 1:
                nc.vector.reduce_max(
                    out=smax[:, t : t + 1], in_=csW[:, 0:G], axis=AX.X
                )
            else:
                if SPLITL > 0:
                    nc.vector.reduce_max(
                        out=smax[:, t : t + 1], in_=csW[:, 0:SPLITL],
                        axis=AX.X,
                    )
                else:
                    nc.vector.memset(smax[:, t : t + 1], -3.0e38)
                if SPLITL < G:
                    nc.vector.reduce_max(
                        out=smax[:, t + 1 : t + 2],
                        in_=csW[:, SPLITL:G], axis=AX.X,
                    )
                else:
                    nc.vector.memset(smax[:, t + 1 : t + 2], -3.0e38)
        nc.sync.dma_start(out=res[:], in_=smax[:])

    nc.compile()
    return nc


MM_DTYPE = "f8"


def _get_nc(w: int):
    key = (w, MM_DTYPE)
    nc = _NC_CACHE.get(key)
    if nc is None:
        nc = _build(w, MM_DTYPE)
        _NC_CACHE[key] = nc
    return nc


def _mm_cast(a: np.ndarray) -> np.ndarray:
    """Convert to the matmul input dtype (host-side rounding)."""
    import ml_dtypes

    if MM_DTYPE == "f8":
        return a.astype(ml_dtypes.float8_e4m3)
    return a.astype(np.float16)


def _prep_inputs(x, w1, b1, queries, values, w):
    """Host-side packing: pad + scale + round + transpose into DMA-friendly
    layouts. Returns the per-core in_maps for run_bass_kernel_spmd."""
    NG = -(-(TPC + w - 1) // G)
    HALO = NG > GPC
    LW = min(G, ((w - 1 + 63) // 64) * 64) if HALO else G
    NGG = (N_CORES - 1) * GPC + NG  # distinct global groups incl. final halo
    w1_scale = W1_SCALE if MM_DTYPE == "f8" else 1.0
    qv_scale = QV_SCALE if MM_DTYPE == "f8" else 1.0
    xpad = np.zeros((NGG * G, D_MODEL), dtype=np.float32)
    xpad[:N_TOKENS] = x
    xr = _mm_cast(xpad)
    # [gg, p, d, t] = xpad[gg*G + t, d*128 + p]
    xg_all = np.ascontiguousarray(
        xr.reshape(NGG, G, ND, P).transpose(0, 3, 2, 1)
    )
    # Column-major pair permutation matching _build's processing order.
    perm = [
        g
        for c in COL_ORDER
        for i in range(NSUB // 2)
        for g in (2 * i * GPS + c, (2 * i + 1) * GPS + c)
    ]
    w1p = np.ascontiguousarray(
        _mm_cast(w1 * w1_scale).reshape(ND, P, D_HID).transpose(1, 0, 2)
    )
    b1p = np.ascontiguousarray(np.asarray(b1, np.float32).reshape(NH2, P).T)
    # Combined probe weights: [k, hh, m] with columns 0..15 = queries.T
    # chunk, columns 16..31 = values.T chunk.
    qv = np.concatenate(
        [np.asarray(queries, np.float32), np.asarray(values, np.float32)], axis=0
    )  # [32, 256]
    qvT = _mm_cast(qv * qv_scale).T.reshape(NH2, P, 2 * N_HEADS)  # [hh, k, m]
    qvp = np.ascontiguousarray(qvT.transpose(1, 0, 2))
    in_maps = []
    for c in range(N_CORES):
        body = xg_all[c * GPC : c * GPC + GPC][perm]  # [32, P, ND, G]
        m = {
            "xp": np.ascontiguousarray(
                body.reshape(NPAIR, 2, P, ND, G).transpose(0, 2, 1, 3, 4)
            ),
            "w1p": w1p,
            "b1p": b1p,
            "qvp": qvp,
        }
        if HALO:
            m["xh"] = np.ascontiguousarray(xg_all[c * GPC + NG - 1, :, :, 0:LW])
        in_maps.append(m)
    return in_maps


def _combine(results, w):
    """Host-side final reduction: per-core [128, GPS+1] -> scalar."""
    qv_scale = QV_SCALE if MM_DTYPE == "f8" else 1.0
    best = np.full(N_HEADS, -np.inf, dtype=np.float64)
    for c in range(N_CORES):
        r = np.asarray(results[c]["res"], dtype=np.float64).reshape(
            NSUB, N_HEADS, GPS + 1
        )
        if c == N_CORES - 1 and w >= 2:
            r = r.copy()
            r[NSUB - 1, :, GPS] = -np.inf  # windows past n - w on last core
        best = np.maximum(best, r.max(axis=(0, 2)))
    return np.asarray(best.sum() / qv_scale, dtype=np.float32)


def kernel(x, w1, b1, queries, values, window_size):
    from concourse.bass_utils import run_bass_kernel_spmd

    x = np.asarray(x, dtype=np.float32)
    w1 = np.asarray(w1, dtype=np.float32)
    b1 = np.asarray(b1, dtype=np.float32)
    queries = np.asarray(queries, dtype=np.float32)
    values = np.asarray(values, dtype=np.float32)
    w = int(np.asarray(window_size))
    assert x.shape == (N_TOKENS, D_MODEL), x.shape
    assert 1 <= w <= G + 1  # slab/halo duplication reads at most one column

    nc = _get_nc(w)
    in_maps = _prep_inputs(x, w1, b1, queries, values, w)
    last_err = None
    for attempt in range(3):
        try:
            out = run_bass_kernel_spmd(nc, in_maps, core_ids=list(range(N_CORES)))
            return _combine(out.results, w)
        except Exception as e:  # transient terminal/device failures
            last_err = e
            import time as _time

            _time.sleep(5.0 * (attempt + 1))
    raise last_err


# Optional: expose a traced run for profiling from test harnesses.
def kernel_traced(x, w1, b1, queries, values, window_size, tmpdir=None):
    from concourse.bass_utils import run_bass_kernel_spmd

    w = int(np.asarray(window_size))
    nc = _get_nc(w)
    in_maps = _prep_inputs(
        np.asarray(x, np.float32),
        np.asarray(w1, np.float32),
        np.asarray(b1, np.float32),
        np.asarray(queries, np.float32),
        np.asarray(values, np.float32),
        w,
    )
    out = run_bass_kernel_spmd(
        nc, in_maps, core_ids=list(range(N_CORES)), trace=True, tmpdir=tmpdir
    )
    return _combine(out.results, w), out
